# revision 19
# baseline (speedup 1.0000x reference)
"""Trainium2 Bass kernel for the CSTR (evaporator) 1M-step scan.

Parallel-in-time: the per-step map is contractive (slow mode ~0.9665/step),
so the trajectory is split into 1024 segments (8 cores x 128 lanes) of
L=1024 steps, each extended K=192 steps back ("spin-up") so an arbitrary
segment-entry state converges below tolerance before the graded region.
Within each lane's window the nonlinear recurrence

  x0' = x0*(SC(u0) - c02*x0 - c03*x1) + c01
  x1' = SA(u1)*x1 + a10*x0 + SB(u0,u1)

is solved by 2 Picard-Gauss-Seidel sweeps (second sweep re-scans from
column 64). States are rescaled (Y0 = x0/c01, Y1 = x1/(a10*c01)) so the
Y0-scan additive term is the constant 1.0 and the Y1-scan additive term
becomes c = Y0 + SBpa + rec_n, which the TENSOR engine accumulates in
PSUM via identity matmuls (fp32, exact) — the Y1 scans read their data1
operand directly from PSUM. The vector engine runs only the 4 linear
scans (tensor_tensor_scan), the reciprocal, and the sweep-2 coefficient
links; all affine precompute runs on the scalar (ACT) engine. Everything
is pipelined in column chunks (PSUM accumulation pieces never cross the
512-col bank boundary). Input DMA is chunked on two DGE queues; outputs
stream out per chunk and are unscaled on host. The first L rows are
computed on host (segment 0 has no spin-up). All param-derived scalars
are per-partition [128,1] operands, so the compiled program is
input-independent.
"""

import numpy as np

T = 1048576
P = 128
NCORES = 8
L = 1024          # graded steps per lane
K = 160           # spin-up steps
W = K + L         # window length per lane (1216)
J0 = 32           # sweep-2 restart column
TC = T // NCORES  # steps per core
SLAB = TC + K     # u rows staged per core
NC_CONST = 17

# fixed model constants (match reference.py)
A, B, C_, D, E, F_, G, H = 0.5616, 0.3126, 48.43, 0.507, 55.0, 0.1538, 90.0, 0.16

# column chunking
CH_TILE = [(0, 192), (192, 832), (832, 1184)]               # DMA staging tiles
CH_DMA = [(0, 192), (192, 512), (512, 832), (832, 1184)]    # precompute grid
CH_A = [(0, 192), (192, 512), (512, 832), (832, 1183)]      # sweep-1 Y0 scan
CH_B = [(0, 512), (512, 1024), (1024, 1183)]                # sweep-1 Y1 scan
CH_VA = [(32, 512), (512, 1024), (1024, 1183)]              # v/a2 links
CH_V = [(32, 512), (512, 1024), (1024, 1183)]               # scanC/scanD
PC1 = [(0, 192), (192, 512), (512, 832), (832, 1024), (1024, 1183)]
PC2 = [(32, 512), (512, 1024), (1024, 1183)]
# X-column ranges streamed to output after sweep-2 scan chunks
OUT_CH = [(160, 513), (513, 1025), (1025, 1184)]

_cache = {}


def _build_nc():
    if "nc" in _cache:
        return _cache["nc"]
    from contextlib import ExitStack
    import concourse.bacc as bacc
    import concourse.tile as tile
    import concourse.mybir as mybir
    from bass_rust import AP

    f32 = mybir.dt.float32
    op = mybir.AluOpType
    ident = mybir.ActivationFunctionType.Identity
    nc = bacc.Bacc("TRN2", target_bir_lowering=False, debug=False,
                   enable_asserts=True, num_devices=NCORES)

    f16 = mybir.dt.float16
    uslab = nc.dram_tensor("uslab", [SLAB, 2], f16, kind="ExternalInput").ap()
    cons = nc.dram_tensor("cons", [P, NC_CONST], f32, kind="ExternalInput").ap()
    iden = nc.dram_tensor("iden", [P, P], f32, kind="ExternalInput").ap()
    o0 = nc.dram_tensor("o0", [P, L], f32, kind="ExternalOutput").ap()
    o1 = nc.dram_tensor("o1", [P, L], f32, kind="ExternalOutput").ap()

    with tile.TileContext(nc) as tc, ExitStack() as ctx:
        pool = ctx.enter_context(tc.tile_pool(name="main", bufs=1))
        ppool = ctx.enter_context(tc.tile_pool(name="psum", bufs=1, space="PSUM"))
        t_u = [pool.tile([P, 2 * (hi - lo)], f16, name=f"u{d}", tag=f"u{d}")
               for d, (lo, hi) in enumerate(CH_TILE)]
        t_cons = pool.tile([P, NC_CONST], f32, name="cons", tag="cons")
        t_iden = pool.tile([P, P], f32, name="iden", tag="iden")
        t_scr = pool.tile([P, 8], f32, name="scr", tag="scr")

        def cst(i):
            return t_cons[:, i : i + 1]

        t_a1 = pool.tile([P, W], f32, name="a1", tag="a1")
        t_den = pool.tile([P, W], f32, name="den", tag="den")
        t_rec = pool.tile([P, W], f32, name="rec", tag="rec")
        t_SA = pool.tile([P, W], f32, name="SA", tag="SA")
        t_SC = pool.tile([P, W], f32, name="SC", tag="SC")
        t_SBp = pool.tile([P, W], f32, name="SBp", tag="SBp")
        t_b = pool.tile([P, W], f32, name="b", tag="b")
        t_v = pool.tile([P, W], f32, name="v", tag="v")
        t_a2 = pool.tile([P, W], f32, name="a2", tag="a2")
        t_Y0a = pool.tile([P, W], f32, name="Y0a", tag="Y0a")
        t_Y1a = pool.tile([P, W], f32, name="Y1a", tag="Y1a")
        t_Y0b = pool.tile([P, W], f32, name="Y0b", tag="Y0b")
        t_Y1b = pool.tile([P, W], f32, name="Y1b", tag="Y1b")
        t_c1p = ppool.tile([P, W - 1], f32, name="c1p", tag="c1p")
        t_c2p = ppool.tile([P, W - 1], f32, name="c2p", tag="c2p")

        def _utile(lo, hi):
            for d, (Lo, Hi) in enumerate(CH_TILE):
                if lo >= Lo and hi <= Hi:
                    return d, Lo
            raise AssertionError((lo, hi))

        def u0q(c):
            lo, hi = CH_DMA[c]
            d, Lo = _utile(lo, hi)
            return t_u[d][:, 2 * (lo - Lo) : 2 * (hi - Lo) : 2]

        def u1q(c):
            lo, hi = CH_DMA[c]
            d, Lo = _utile(lo, hi)
            return t_u[d][:, 2 * (lo - Lo) + 1 : 2 * (hi - Lo) : 2]

        # ---- preamble: DMA issue + engine warms --------------------------
        nc.gpsimd.memset(t_scr[:, 0:4], 0.0)
        nc.scalar.activation(t_scr[:, 0:1], t_scr[:, 1:2], ident,
                             bias=0.0, scale=1.0)
        # b tile (constant 1.0) built before the Pool-queue SWDGE work
        nc.gpsimd.memset(t_b[:], 1.0)
        nc.sync.dma_start(t_cons[:], cons[:])

        # input tiles (fp16) partition-split across the three DGE queues
        def dma_half(d, half, eng):
            lo, hi = CH_TILE[d]
            w2 = 2 * (hi - lo)
            off = half * 64 * L * 2 + 2 * lo
            win = AP(uslab.tensor, off, [[L * 2, 64], [1, w2]])
            eng.dma_start(t_u[d][64 * half : 64 * (half + 1), :], win)

        dma_half(0, 0, nc.sync)
        dma_half(0, 1, nc.scalar)
        dma_half(2, 0, nc.gpsimd)
        dma_half(1, 0, nc.sync)
        nc.scalar.dma_start(t_iden[:], iden[:])
        dma_half(1, 1, nc.scalar)
        dma_half(2, 1, nc.sync)
        # Y0a column 0 = i0/c01 (read by the c1 PSUM accumulation)
        nc.scalar.activation(t_Y0a[:, 0:1], cst(15), ident, bias=0.0, scale=1.0)

        # ---- op builders -------------------------------------------------
        def act_pre(d):
            lo, hi = CH_DMA[d]
            nc.scalar.activation(t_a1[:, lo:hi], u0q(d), ident,
                                 bias=cst(1), scale=cst(0))
            nc.scalar.activation(t_den[:, lo:hi], u1q(d), ident,
                                 bias=cst(3), scale=cst(2))

        def act_post(d):
            lo, hi = CH_DMA[d]
            nc.scalar.activation(t_SA[:, lo:hi], t_rec[:, lo:hi], ident,
                                 bias=cst(5), scale=cst(4))
            nc.scalar.activation(t_SC[:, lo:hi], u0q(d), ident,
                                 bias=cst(7), scale=cst(6))
            nc.scalar.activation(t_SBp[:, lo:hi], u0q(d), ident,
                                 bias=cst(9), scale=cst(8))

        def rec(d):
            lo, hi = CH_DMA[d]
            nc.vector.reciprocal_approx_fast(t_rec[:, lo:hi], t_den[:, lo:hi])

        def mmSB(tp, lo, hi):
            # PSUM accumulation: tp[lo:hi] = SBpa + rec_n  (both +I matmuls)
            nc.tensor.matmul(tp[:, lo:hi], t_iden[:], t_SBp[:, lo:hi],
                             start=True, stop=False)
            nc.tensor.matmul(tp[:, lo:hi], t_iden[:], t_rec[:, lo:hi],
                             start=False, stop=False)

        def mmY(tp, src, lo, hi):
            # tp[lo:hi] += Y0 scan output (closes the accumulation group)
            nc.tensor.matmul(tp[:, lo:hi], t_iden[:], src[:, lo:hi],
                             start=False, stop=True)

        def v_(e):  # v = -c02*c01*Y0a + SC   (DVE stt)
            lo, hi = CH_VA[e]
            nc.vector.scalar_tensor_tensor(t_v[:, lo:hi], t_Y0a[:, lo:hi],
                                           cst(13), t_SC[:, lo:hi],
                                           op.mult, op.add)

        def a2_(e):  # a2 = -c03*al*Y1a + v   (DVE stt)
            lo, hi = CH_VA[e]
            nc.vector.scalar_tensor_tensor(t_a2[:, lo:hi], t_Y1a[:, lo:hi],
                                           cst(14), t_v[:, lo:hi],
                                           op.mult, op.add)

        def scanA(d):  # sweep-1 Y0
            lo, hi = CH_A[d]
            init = cst(15) if d == 0 else t_Y0a[:, lo : lo + 1]
            nc.vector.tensor_tensor_scan(t_Y0a[:, lo + 1 : hi + 1],
                                         t_a1[:, lo:hi], t_b[:, lo:hi],
                                         init, op.mult, op.add)

        def scanB(d):  # sweep-1 Y1 (data1 from PSUM)
            lo, hi = CH_B[d]
            init = cst(16) if d == 0 else t_Y1a[:, lo : lo + 1]
            nc.vector.tensor_tensor_scan(t_Y1a[:, lo + 1 : hi + 1],
                                         t_SA[:, lo:hi], t_c1p[:, lo:hi],
                                         init, op.mult, op.add)

        def scanC(e):  # sweep-2 Y0
            lo, hi = CH_V[e]
            init = t_Y0a[:, lo : lo + 1] if e == 0 else t_Y0b[:, lo : lo + 1]
            nc.vector.tensor_tensor_scan(t_Y0b[:, lo + 1 : hi + 1],
                                         t_a2[:, lo:hi], t_b[:, lo:hi],
                                         init, op.mult, op.add)

        def scanD(e):  # sweep-2 Y1 (data1 from PSUM)
            lo, hi = CH_V[e]
            init = t_Y1a[:, lo : lo + 1] if e == 0 else t_Y1b[:, lo : lo + 1]
            nc.vector.tensor_tensor_scan(t_Y1b[:, lo + 1 : hi + 1],
                                         t_SA[:, lo:hi], t_c2p[:, lo:hi],
                                         init, op.mult, op.add)

        def out0(i):
            lo, hi = OUT_CH[i]
            nc.sync.dma_start(o0[:, lo - K : hi - K], t_Y0b[:, lo:hi])

        def out1(i):
            lo, hi = OUT_CH[i]
            if i == len(OUT_CH) - 1:
                # final chunk split across both queues to shorten the tail
                nc.scalar.dma_start(o1[0:64, lo - K : hi - K],
                                    t_Y1b[0:64, lo:hi])
                nc.sync.dma_start(o1[64:128, lo - K : hi - K],
                                  t_Y1b[64:128, lo:hi])
            else:
                nc.scalar.dma_start(o1[:, lo - K : hi - K], t_Y1b[:, lo:hi])

        def copy64():  # Y0b col 64 = Y0a col 64 (read by the c2 accumulation)
            nc.scalar.activation(t_Y0b[:, J0 : J0 + 1], t_Y0a[:, J0 : J0 + 1],
                                 ident, bias=0.0, scale=1.0)

        # ---- pipelined emission ------------------------------------------
        act_pre(0)
        rec(0)
        act_pre(1)
        act_post(0)
        mmSB(t_c1p, *PC1[0])
        scanA(0)
        rec(1)
        act_pre(2)
        act_post(1)
        mmY(t_c1p, t_Y0a, *PC1[0])
        mmSB(t_c1p, *PC1[1])
        scanA(1)
        rec(2)
        act_pre(3)
        act_post(2)
        copy64()
        mmY(t_c1p, t_Y0a, *PC1[1])
        mmSB(t_c1p, *PC1[2])
        scanB(0)
        scanA(2)
        mmY(t_c1p, t_Y0a, *PC1[2])
        rec(3)
        act_post(3)
        mmSB(t_c1p, *PC1[3])
        mmSB(t_c1p, *PC1[4])
        scanA(3)
        mmY(t_c1p, t_Y0a, *PC1[3])
        mmY(t_c1p, t_Y0a, *PC1[4])
        scanB(1)
        v_(0)
        a2_(0)
        mmSB(t_c2p, *PC2[0])
        scanC(0)
        mmY(t_c2p, t_Y0b, *PC2[0])
        scanB(2)
        v_(1)
        a2_(1)
        scanD(0)
        mmSB(t_c2p, *PC2[1])
        scanC(1)
        mmY(t_c2p, t_Y0b, *PC2[1])
        out0(0)
        v_(2)
        a2_(2)
        scanD(1)
        out1(0)
        scanC(2)
        mmSB(t_c2p, *PC2[2])
        mmY(t_c2p, t_Y0b, *PC2[2])
        out0(1)
        scanD(2)
        out1(1)
        out0(2)
        out1(2)

    nc.compile()
    _cache["nc"] = nc
    return nc


def _derive(params, x0):
    M, Cc, UA2, Cp, lam, lams, F1, X1p, F3, T1, T200 = [float(params[i]) for i in range(11)]
    UA1 = H * (F1 + F3)
    k1 = (UA1 + F1 * Cp) / lam
    p_ = k1 * B
    q_ = k1 * A
    alpha_u = UA1 * F_ / lam
    alpha_c = (UA1 * G + F1 * Cp * T1) / lam - k1 * C_
    c01 = F1 * X1p / M
    c02 = p_ / M
    c03 = q_ / M
    a10 = -p_ / Cc
    cA2 = -D / (lam * Cc)
    cA1 = 1.0 - q_ / Cc
    cB2 = alpha_u / Cc
    cB1 = alpha_c / Cc
    cB3 = -(E - T200) / (lam * Cc)
    cC2 = alpha_u / M
    cC1 = 1.0 - (F1 - alpha_c) / M
    i0, i1 = float(x0[0]), float(x0[1])
    al = a10 * c01                 # alpha (< 0)
    s_ = -cB3 * UA2 * UA2          # > 0

    cv = np.zeros(NC_CONST, np.float64)
    cv[0] = cC2                           # a1 scale
    cv[1] = cC1 - (c02 * i0 + c03 * i1)   # a1 bias
    cv[2] = 2.0 * Cp * al / s_            # den_n scale (negative)
    cv[3] = UA2 * al / s_                 # den_n bias (negative)
    cv[4] = -cA2 * UA2 * UA2 * al / s_    # SA scale (of rec_n)
    cv[5] = cA1 + cA2 * UA2               # SA bias
    cv[6] = cC2                           # SC scale
    cv[7] = cC1                           # SC bias
    cv[8] = cB2 / al                      # SBpa scale
    cv[9] = (cB1 + cB3 * UA2) / al        # SBpa bias
    cv[13] = -c02 * c01                   # v scalar
    cv[14] = -c03 * al                    # a2 scalar
    cv[15] = i0 / c01
    cv[16] = i1 / al
    return cv.astype(np.float32), np.float32(c01), np.float32(al)


def _make_in_maps(u, cons):
    u = np.ascontiguousarray(u, np.float32)
    cons = np.tile(cons[None, :], (P, 1))
    eye = np.eye(P, dtype=np.float32)
    in_maps = []
    for c in range(NCORES):
        if c == 0:
            slab = np.concatenate([np.repeat(u[0:1], K, axis=0), u[0:TC]], axis=0)
        else:
            slab = u[c * TC - K : c * TC + TC]
        in_maps.append({
            "uslab": np.ascontiguousarray(slab, np.float16),
            "cons": cons,
            "iden": eye,
        })
    return in_maps


def _host_head(u, x0, params, n):
    # exact fp32 simulation of the first n steps (segment 0 has no spin-up)
    f = np.float32
    M, Cc, UA2, Cp, lam, lams, F1, X1p, F3, T1, T200 = [f(params[i]) for i in range(11)]
    out = np.empty((n, 2), f)
    s0, s1 = f(x0[0]), f(x0[1])
    fA, fB, fC, fD, fE, fF, fG, fH = f(A), f(B), f(C_), f(D), f(E), f(F_), f(G), f(H)
    one, two = f(1.0), f(2.0)
    UA1 = fH * (F1 + F3)
    for t in range(n):
        out[t, 0] = s0
        out[t, 1] = s1
        u0, u1 = f(u[t, 0]), f(u[t, 1])
        T2 = fA * s1 + fB * s0 + fC
        T3 = fD * s1 + fE
        T100 = fF * u0 + fG
        Q100 = UA1 * (T100 - T2)
        Q200 = UA2 * (T3 - T200) / (one + UA2 / (two * Cp * u1))
        F5 = Q200 / lam
        F4 = (Q100 - F1 * Cp * (T2 - T1)) / lam
        F2 = F1 - F4
        X2d = (F1 * X1p - F2 * s0) / M
        P2d = (F4 - F5) / Cc
        s0 = s0 + X2d
        s1 = s1 + P2d
    return out


def _assemble(results, head, c01, al):
    out = np.empty((T, 2), np.float32)
    for c in range(NCORES):
        out[c * TC : (c + 1) * TC, 0] = results[c]["o0"].reshape(-1) * c01
        out[c * TC : (c + 1) * TC, 1] = results[c]["o1"].reshape(-1) * al
    out[0:L] = head
    return out


def run(u_forced, x0, params, trace=False):
    from concourse.bass_utils import run_bass_kernel_spmd
    nc = _build_nc()
    cons, c01, al = _derive(params, x0)
    in_maps = _make_in_maps(u_forced, cons)
    head = _host_head(u_forced, x0, params, L)
    res = run_bass_kernel_spmd(nc, in_maps, list(range(NCORES)), trace=trace)
    return _assemble(res.results, head, c01, al), res


def kernel(u_forced, x0, params):
    out, _ = run(u_forced, x0, params, trace=False)
    return out


# revision 20
# speedup vs baseline: 1.0472x; 1.0472x over previous
"""Trainium2 Bass kernel for the CSTR (evaporator) 1M-step scan.

Parallel-in-time: the per-step map is contractive (slow mode ~0.9665/step),
so the trajectory is split into 1024 segments (8 cores x 128 lanes) of
L=1024 steps, each extended K=192 steps back ("spin-up") so an arbitrary
segment-entry state converges below tolerance before the graded region.
Within each lane's window the nonlinear recurrence

  x0' = x0*(SC(u0) - c02*x0 - c03*x1) + c01
  x1' = SA(u1)*x1 + a10*x0 + SB(u0,u1)

is solved by 2 Picard-Gauss-Seidel sweeps (second sweep re-scans from
column 64). States are rescaled (Y0 = x0/c01, Y1 = x1/(a10*c01)) so the
Y0-scan additive term is the constant 1.0 and the Y1-scan additive term
becomes c = Y0 + SBpa + rec_n, which the TENSOR engine accumulates in
PSUM via identity matmuls (fp32, exact) — the Y1 scans read their data1
operand directly from PSUM. The vector engine runs only the 4 linear
scans (tensor_tensor_scan), the reciprocal, and the sweep-2 coefficient
links; all affine precompute runs on the scalar (ACT) engine. Everything
is pipelined in column chunks (PSUM accumulation pieces never cross the
512-col bank boundary). Input DMA is chunked on two DGE queues; outputs
stream out per chunk and are unscaled on host. The first L rows are
computed on host (segment 0 has no spin-up). All param-derived scalars
are per-partition [128,1] operands, so the compiled program is
input-independent.
"""

import numpy as np

T = 1048576
P = 128
NCORES = 8
L = 1024          # graded steps per lane
K = 192           # spin-up steps
W = K + L         # window length per lane (1216)
J0 = 64           # sweep-2 restart column
TC = T // NCORES  # steps per core
SLAB = TC + K     # u rows staged per core
NC_CONST = 17

# fixed model constants (match reference.py)
A, B, C_, D, E, F_, G, H = 0.5616, 0.3126, 48.43, 0.507, 55.0, 0.1538, 90.0, 0.16

# column chunking
CH_TILE = [(0, 192), (192, 832), (832, 1216)]               # DMA staging tiles
CH_DMA = [(0, 192), (192, 512), (512, 832), (832, 1216)]    # precompute grid
CH_A = [(0, 192), (192, 512), (512, 832), (832, 1215)]      # sweep-1 Y0 scan
CH_B = [(0, 512), (512, 1024), (1024, 1215)]                # sweep-1 Y1 scan
CH_VA = [(64, 512), (512, 1024), (1024, 1215)]              # v/a2 links
CH_V = [(64, 512), (512, 1024), (1024, 1215)]               # scanC/scanD
PC1 = [(0, 192), (192, 512), (512, 832), (832, 1024), (1024, 1215)]
PC2 = [(64, 512), (512, 1024), (1024, 1215)]
# X-column ranges streamed to output after sweep-2 scan chunks
OUT_CH = [(192, 513), (513, 1025), (1025, 1216)]

_cache = {}


def _build_nc():
    if "nc" in _cache:
        return _cache["nc"]
    from contextlib import ExitStack
    import concourse.bacc as bacc
    import concourse.tile as tile
    import concourse.mybir as mybir
    from bass_rust import AP

    f32 = mybir.dt.float32
    op = mybir.AluOpType
    ident = mybir.ActivationFunctionType.Identity
    nc = bacc.Bacc("TRN2", target_bir_lowering=False, debug=False,
                   enable_asserts=True, num_devices=NCORES)

    f16 = mybir.dt.float16
    uslab = nc.dram_tensor("uslab", [SLAB, 2], f16, kind="ExternalInput").ap()
    cons = nc.dram_tensor("cons", [P, NC_CONST], f32, kind="ExternalInput").ap()
    iden = nc.dram_tensor("iden", [P, P], f32, kind="ExternalInput").ap()
    o0 = nc.dram_tensor("o0", [P, L], f32, kind="ExternalOutput").ap()
    o1 = nc.dram_tensor("o1", [P, L], f32, kind="ExternalOutput").ap()

    with tile.TileContext(nc) as tc, ExitStack() as ctx:
        pool = ctx.enter_context(tc.tile_pool(name="main", bufs=1))
        ppool = ctx.enter_context(tc.tile_pool(name="psum", bufs=1, space="PSUM"))
        t_u = [pool.tile([P, 2 * (hi - lo)], f16, name=f"u{d}", tag=f"u{d}")
               for d, (lo, hi) in enumerate(CH_TILE)]
        t_cons = pool.tile([P, NC_CONST], f32, name="cons", tag="cons")
        t_iden = pool.tile([P, P], f32, name="iden", tag="iden")
        t_scr = pool.tile([P, 8], f32, name="scr", tag="scr")

        def cst(i):
            return t_cons[:, i : i + 1]

        t_a1 = pool.tile([P, W], f32, name="a1", tag="a1")
        t_den = pool.tile([P, W], f32, name="den", tag="den")
        t_rec = pool.tile([P, W], f32, name="rec", tag="rec")
        t_SA = pool.tile([P, W], f32, name="SA", tag="SA")
        t_SC = pool.tile([P, W], f32, name="SC", tag="SC")
        t_SBp = pool.tile([P, W], f32, name="SBp", tag="SBp")
        t_b = pool.tile([P, W], f32, name="b", tag="b")
        t_v = pool.tile([P, W], f32, name="v", tag="v")
        t_a2 = pool.tile([P, W], f32, name="a2", tag="a2")
        t_Y0a = pool.tile([P, W], f32, name="Y0a", tag="Y0a")
        t_Y1a = pool.tile([P, W], f32, name="Y1a", tag="Y1a")
        t_Y0b = pool.tile([P, W], f32, name="Y0b", tag="Y0b")
        t_Y1b = pool.tile([P, W], f32, name="Y1b", tag="Y1b")
        t_c1p = ppool.tile([P, W - 1], f32, name="c1p", tag="c1p")
        t_c2p = ppool.tile([P, W - 1], f32, name="c2p", tag="c2p")

        def _utile(lo, hi):
            for d, (Lo, Hi) in enumerate(CH_TILE):
                if lo >= Lo and hi <= Hi:
                    return d, Lo
            raise AssertionError((lo, hi))

        def u0q(c):
            lo, hi = CH_DMA[c]
            d, Lo = _utile(lo, hi)
            return t_u[d][:, 2 * (lo - Lo) : 2 * (hi - Lo) : 2]

        def u1q(c):
            lo, hi = CH_DMA[c]
            d, Lo = _utile(lo, hi)
            return t_u[d][:, 2 * (lo - Lo) + 1 : 2 * (hi - Lo) : 2]

        # ---- preamble: DMA issue + engine warms --------------------------
        nc.gpsimd.memset(t_scr[:, 0:4], 0.0)
        nc.scalar.activation(t_scr[:, 0:1], t_scr[:, 1:2], ident,
                             bias=0.0, scale=1.0)
        # b tile (constant 1.0) built before the Pool-queue SWDGE work
        nc.gpsimd.memset(t_b[:], 1.0)
        nc.sync.dma_start(t_cons[:], cons[:])

        # input tiles (fp16) partition-split across the three DGE queues
        def dma_half(d, half, eng):
            lo, hi = CH_TILE[d]
            w2 = 2 * (hi - lo)
            off = half * 64 * L * 2 + 2 * lo
            win = AP(uslab.tensor, off, [[L * 2, 64], [1, w2]])
            eng.dma_start(t_u[d][64 * half : 64 * (half + 1), :], win)

        dma_half(0, 0, nc.sync)
        dma_half(0, 1, nc.scalar)
        dma_half(2, 0, nc.gpsimd)
        dma_half(1, 0, nc.sync)
        nc.scalar.dma_start(t_iden[:], iden[:])
        dma_half(1, 1, nc.scalar)
        dma_half(2, 1, nc.sync)
        # Y0a column 0 = i0/c01 (read by the c1 PSUM accumulation)
        nc.scalar.activation(t_Y0a[:, 0:1], cst(15), ident, bias=0.0, scale=1.0)

        # ---- op builders -------------------------------------------------
        def act_pre(d):
            lo, hi = CH_DMA[d]
            nc.scalar.activation(t_a1[:, lo:hi], u0q(d), ident,
                                 bias=cst(1), scale=cst(0))
            nc.scalar.activation(t_den[:, lo:hi], u1q(d), ident,
                                 bias=cst(3), scale=cst(2))

        def act_post(d):
            lo, hi = CH_DMA[d]
            nc.scalar.activation(t_SA[:, lo:hi], t_rec[:, lo:hi], ident,
                                 bias=cst(5), scale=cst(4))
            nc.scalar.activation(t_SC[:, lo:hi], u0q(d), ident,
                                 bias=cst(7), scale=cst(6))
            nc.scalar.activation(t_SBp[:, lo:hi], u0q(d), ident,
                                 bias=cst(9), scale=cst(8))

        def rec(d):
            lo, hi = CH_DMA[d]
            nc.vector.reciprocal_approx_fast(t_rec[:, lo:hi], t_den[:, lo:hi])

        def mmSB(tp, lo, hi):
            # PSUM accumulation: tp[lo:hi] = SBpa + rec_n  (both +I matmuls)
            nc.tensor.matmul(tp[:, lo:hi], t_iden[:], t_SBp[:, lo:hi],
                             start=True, stop=False)
            nc.tensor.matmul(tp[:, lo:hi], t_iden[:], t_rec[:, lo:hi],
                             start=False, stop=False)

        def mmY(tp, src, lo, hi):
            # tp[lo:hi] += Y0 scan output (closes the accumulation group)
            nc.tensor.matmul(tp[:, lo:hi], t_iden[:], src[:, lo:hi],
                             start=False, stop=True)

        def v_(e):  # v = -c02*c01*Y0a + SC   (DVE stt)
            lo, hi = CH_VA[e]
            nc.vector.scalar_tensor_tensor(t_v[:, lo:hi], t_Y0a[:, lo:hi],
                                           cst(13), t_SC[:, lo:hi],
                                           op.mult, op.add)

        def a2_(e):  # a2 = -c03*al*Y1a + v   (DVE stt)
            lo, hi = CH_VA[e]
            nc.vector.scalar_tensor_tensor(t_a2[:, lo:hi], t_Y1a[:, lo:hi],
                                           cst(14), t_v[:, lo:hi],
                                           op.mult, op.add)

        def scanA(d):  # sweep-1 Y0
            lo, hi = CH_A[d]
            init = cst(15) if d == 0 else t_Y0a[:, lo : lo + 1]
            nc.vector.tensor_tensor_scan(t_Y0a[:, lo + 1 : hi + 1],
                                         t_a1[:, lo:hi], t_b[:, lo:hi],
                                         init, op.mult, op.add)

        def scanB(d):  # sweep-1 Y1 (data1 from PSUM)
            lo, hi = CH_B[d]
            init = cst(16) if d == 0 else t_Y1a[:, lo : lo + 1]
            nc.vector.tensor_tensor_scan(t_Y1a[:, lo + 1 : hi + 1],
                                         t_SA[:, lo:hi], t_c1p[:, lo:hi],
                                         init, op.mult, op.add)

        def scanC(e):  # sweep-2 Y0
            lo, hi = CH_V[e]
            init = t_Y0a[:, lo : lo + 1] if e == 0 else t_Y0b[:, lo : lo + 1]
            nc.vector.tensor_tensor_scan(t_Y0b[:, lo + 1 : hi + 1],
                                         t_a2[:, lo:hi], t_b[:, lo:hi],
                                         init, op.mult, op.add)

        def scanD(e):  # sweep-2 Y1 (data1 from PSUM)
            lo, hi = CH_V[e]
            init = t_Y1a[:, lo : lo + 1] if e == 0 else t_Y1b[:, lo : lo + 1]
            nc.vector.tensor_tensor_scan(t_Y1b[:, lo + 1 : hi + 1],
                                         t_SA[:, lo:hi], t_c2p[:, lo:hi],
                                         init, op.mult, op.add)

        def out0(i):
            lo, hi = OUT_CH[i]
            nc.sync.dma_start(o0[:, lo - K : hi - K], t_Y0b[:, lo:hi])

        def out1(i):
            lo, hi = OUT_CH[i]
            if i == len(OUT_CH) - 1:
                # final chunk split across both queues to shorten the tail
                nc.scalar.dma_start(o1[0:64, lo - K : hi - K],
                                    t_Y1b[0:64, lo:hi])
                nc.sync.dma_start(o1[64:128, lo - K : hi - K],
                                  t_Y1b[64:128, lo:hi])
            else:
                nc.scalar.dma_start(o1[:, lo - K : hi - K], t_Y1b[:, lo:hi])

        def copy64():  # Y0b col 64 = Y0a col 64 (read by the c2 accumulation)
            nc.scalar.activation(t_Y0b[:, J0 : J0 + 1], t_Y0a[:, J0 : J0 + 1],
                                 ident, bias=0.0, scale=1.0)

        # ---- pipelined emission ------------------------------------------
        act_pre(0)
        rec(0)
        act_pre(1)
        act_post(0)
        mmSB(t_c1p, *PC1[0])
        scanA(0)
        rec(1)
        act_pre(2)
        act_post(1)
        mmY(t_c1p, t_Y0a, *PC1[0])
        mmSB(t_c1p, *PC1[1])
        scanA(1)
        rec(2)
        act_pre(3)
        act_post(2)
        copy64()
        mmY(t_c1p, t_Y0a, *PC1[1])
        mmSB(t_c1p, *PC1[2])
        scanB(0)
        scanA(2)
        mmY(t_c1p, t_Y0a, *PC1[2])
        rec(3)
        act_post(3)
        mmSB(t_c1p, *PC1[3])
        mmSB(t_c1p, *PC1[4])
        scanA(3)
        mmY(t_c1p, t_Y0a, *PC1[3])
        mmY(t_c1p, t_Y0a, *PC1[4])
        scanB(1)
        v_(0)
        a2_(0)
        mmSB(t_c2p, *PC2[0])
        scanC(0)
        mmY(t_c2p, t_Y0b, *PC2[0])
        scanB(2)
        v_(1)
        a2_(1)
        scanD(0)
        mmSB(t_c2p, *PC2[1])
        scanC(1)
        mmY(t_c2p, t_Y0b, *PC2[1])
        out0(0)
        v_(2)
        a2_(2)
        scanD(1)
        out1(0)
        scanC(2)
        mmSB(t_c2p, *PC2[2])
        mmY(t_c2p, t_Y0b, *PC2[2])
        out0(1)
        scanD(2)
        out1(1)
        out0(2)
        out1(2)

    nc.compile()
    _cache["nc"] = nc
    return nc


def _derive(params, x0):
    M, Cc, UA2, Cp, lam, lams, F1, X1p, F3, T1, T200 = [float(params[i]) for i in range(11)]
    UA1 = H * (F1 + F3)
    k1 = (UA1 + F1 * Cp) / lam
    p_ = k1 * B
    q_ = k1 * A
    alpha_u = UA1 * F_ / lam
    alpha_c = (UA1 * G + F1 * Cp * T1) / lam - k1 * C_
    c01 = F1 * X1p / M
    c02 = p_ / M
    c03 = q_ / M
    a10 = -p_ / Cc
    cA2 = -D / (lam * Cc)
    cA1 = 1.0 - q_ / Cc
    cB2 = alpha_u / Cc
    cB1 = alpha_c / Cc
    cB3 = -(E - T200) / (lam * Cc)
    cC2 = alpha_u / M
    cC1 = 1.0 - (F1 - alpha_c) / M
    i0, i1 = float(x0[0]), float(x0[1])
    al = a10 * c01                 # alpha (< 0)
    s_ = -cB3 * UA2 * UA2          # > 0

    cv = np.zeros(NC_CONST, np.float64)
    cv[0] = cC2                           # a1 scale
    cv[1] = cC1 - (c02 * i0 + c03 * i1)   # a1 bias
    cv[2] = 2.0 * Cp * al / s_            # den_n scale (negative)
    cv[3] = UA2 * al / s_                 # den_n bias (negative)
    cv[4] = -cA2 * UA2 * UA2 * al / s_    # SA scale (of rec_n)
    cv[5] = cA1 + cA2 * UA2               # SA bias
    cv[6] = cC2                           # SC scale
    cv[7] = cC1                           # SC bias
    cv[8] = cB2 / al                      # SBpa scale
    cv[9] = (cB1 + cB3 * UA2) / al        # SBpa bias
    cv[13] = -c02 * c01                   # v scalar
    cv[14] = -c03 * al                    # a2 scalar
    cv[15] = i0 / c01
    cv[16] = i1 / al
    return cv.astype(np.float32), np.float32(c01), np.float32(al)


def _make_in_maps(u, cons):
    u = np.ascontiguousarray(u, np.float32)
    cons = np.tile(cons[None, :], (P, 1))
    eye = np.eye(P, dtype=np.float32)
    in_maps = []
    for c in range(NCORES):
        if c == 0:
            slab = np.concatenate([np.repeat(u[0:1], K, axis=0), u[0:TC]], axis=0)
        else:
            slab = u[c * TC - K : c * TC + TC]
        in_maps.append({
            "uslab": np.ascontiguousarray(slab, np.float16),
            "cons": cons,
            "iden": eye,
        })
    return in_maps


def _host_head(u, x0, params, n):
    # exact fp32 simulation of the first n steps (segment 0 has no spin-up)
    f = np.float32
    M, Cc, UA2, Cp, lam, lams, F1, X1p, F3, T1, T200 = [f(params[i]) for i in range(11)]
    out = np.empty((n, 2), f)
    s0, s1 = f(x0[0]), f(x0[1])
    fA, fB, fC, fD, fE, fF, fG, fH = f(A), f(B), f(C_), f(D), f(E), f(F_), f(G), f(H)
    one, two = f(1.0), f(2.0)
    UA1 = fH * (F1 + F3)
    for t in range(n):
        out[t, 0] = s0
        out[t, 1] = s1
        u0, u1 = f(u[t, 0]), f(u[t, 1])
        T2 = fA * s1 + fB * s0 + fC
        T3 = fD * s1 + fE
        T100 = fF * u0 + fG
        Q100 = UA1 * (T100 - T2)
        Q200 = UA2 * (T3 - T200) / (one + UA2 / (two * Cp * u1))
        F5 = Q200 / lam
        F4 = (Q100 - F1 * Cp * (T2 - T1)) / lam
        F2 = F1 - F4
        X2d = (F1 * X1p - F2 * s0) / M
        P2d = (F4 - F5) / Cc
        s0 = s0 + X2d
        s1 = s1 + P2d
    return out


def _assemble(results, head, c01, al):
    out = np.empty((T, 2), np.float32)
    for c in range(NCORES):
        out[c * TC : (c + 1) * TC, 0] = results[c]["o0"].reshape(-1) * c01
        out[c * TC : (c + 1) * TC, 1] = results[c]["o1"].reshape(-1) * al
    out[0:L] = head
    return out


def run(u_forced, x0, params, trace=False):
    from concourse.bass_utils import run_bass_kernel_spmd
    nc = _build_nc()
    cons, c01, al = _derive(params, x0)
    in_maps = _make_in_maps(u_forced, cons)
    head = _host_head(u_forced, x0, params, L)
    res = run_bass_kernel_spmd(nc, in_maps, list(range(NCORES)), trace=trace)
    return _assemble(res.results, head, c01, al), res


def kernel(u_forced, x0, params):
    out, _ = run(u_forced, x0, params, trace=False)
    return out
